# revision 2
# baseline (speedup 1.0000x reference)
"""Distributed causal attention + RoPE for trn2 (8 NeuronCores).

Sharding: batch (2) x head-groups (4 heads/core). Core c: batch c//4,
heads 4*(c%4)..4*(c%4)+3. Attention computed in S^T layout
([k_part, q_free]) so no on-device transposes are needed; softmax sums
come from a ones-vector matmul over partitions. Out-projection is
column-parallel after an intra-group AllGather of the per-core
attention outputs.
"""
import sys
for _p in ('/opt/trn_rl_repo',):
    if _p not in sys.path:
        sys.path.insert(0, _p)

import numpy as np
import ml_dtypes

B, S, H, NH, HD = 2, 2048, 2048, 16, 128
HPC = 4            # heads per core
DH = HPC * HD      # 512 local dims
QC = 512           # q-chunk width (attention + AG round)
SCALE = HD ** -0.5

_cached = {}


def _build():
    import concourse.bacc as bacc
    import concourse.mybir as mybir
    import concourse.tile as tile

    F32 = mybir.dt.float32
    BF = mybir.dt.bfloat16
    AF = mybir.ActivationFunctionType
    ALU = mybir.AluOpType

    nc = bacc.Bacc("TRN2", target_bir_lowering=False, debug=False, num_devices=8)
    xT_d = nc.dram_tensor("xT", [H, S], BF, kind="ExternalInput").ap()
    wqT_d = nc.dram_tensor("wqT", [H, DH], BF, kind="ExternalInput").ap()
    wkT_d = nc.dram_tensor("wkT", [H, DH], BF, kind="ExternalInput").ap()
    wvT_d = nc.dram_tensor("wvT", [H, DH], BF, kind="ExternalInput").ap()
    woT_d = nc.dram_tensor("woT", [H, DH], BF, kind="ExternalInput").ap()
    cosT_d = nc.dram_tensor("cosT", [HD, S], F32, kind="ExternalInput").ap()
    sinTs_d = nc.dram_tensor("sinTs", [HD, S], F32, kind="ExternalInput").ap()
    mask_d = nc.dram_tensor("mask01", [128, 128], BF, kind="ExternalInput").ap()
    out_d = nc.dram_tensor("out", [S, DH], F32, kind="ExternalOutput").ap()

    EB = H // 128     # 16 contraction blocks
    n_sc = S // QC    # 4 s-chunks

    with tile.TileContext(nc) as tc:
        with tc.tile_pool(name="wpool", bufs=3) as wpool, \
             tc.tile_pool(name="wo", bufs=1) as wop, \
             tc.tile_pool(name="xp", bufs=2) as xp, \
             tc.tile_pool(name="consts", bufs=1) as cp, \
             tc.tile_pool(name="qk", bufs=1) as qkp, \
             tc.tile_pool(name="vp", bufs=1) as vp, \
             tc.tile_pool(name="rope", bufs=2) as rp, \
             tc.tile_pool(name="at", bufs=4) as atp, \
             tc.tile_pool(name="ot", bufs=4) as otp, \
             tc.tile_pool(name="rn", bufs=2) as rnp, \
             tc.tile_pool(name="oc", bufs=2) as ocp, \
             tc.tile_pool(name="ppA", bufs=2, space="PSUM") as ppA, \
             tc.tile_pool(name="ppS", bufs=2, space="PSUM") as ppS, \
             tc.tile_pool(name="ppO", bufs=2, space="PSUM") as ppO, \
             tc.tile_pool(name="ppR", bufs=2, space="PSUM") as ppR, \
             tc.tile_pool(name="dramp", bufs=2, space="DRAM") as dramp:

            # ---- constants / weights ----
            wq_sb = wpool.tile([128, EB, DH], BF, tag="w")
            wk_sb = wpool.tile([128, EB, DH], BF, tag="w")
            wv_sb = wpool.tile([128, EB, DH], BF, tag="w")
            nc.sync.dma_start(out=wq_sb[:], in_=wqT_d.rearrange("(e p) d -> p e d", p=128))
            nc.sync.dma_start(out=wk_sb[:], in_=wkT_d.rearrange("(e p) d -> p e d", p=128))
            nc.sync.dma_start(out=wv_sb[:], in_=wvT_d.rearrange("(e p) d -> p e d", p=128))
            wo_sb = wop.tile([128, EB, DH], BF, tag="wo")
            nc.sync.dma_start(out=wo_sb[:], in_=woT_d.rearrange("(e p) d -> p e d", p=128))
            cos_sb = cp.tile([HD, S], F32, tag="cos")
            sin_sb = cp.tile([HD, S], F32, tag="sin")
            nc.sync.dma_start(out=cos_sb[:], in_=cosT_d[:])
            nc.sync.dma_start(out=sin_sb[:], in_=sinTs_d[:])
            mask_sb = cp.tile([128, 128], BF, tag="mask")
            nc.sync.dma_start(out=mask_sb[:], in_=mask_d[:])
            ones_sb = cp.tile([128, 1], BF, tag="ones")
            nc.vector.memset(ones_sb[:], 1.0)

            qT = [qkp.tile([HD, S], BF, tag=f"qT{h}", name=f"qT{h}") for h in range(HPC)]
            kT = [qkp.tile([HD, S], BF, tag=f"kT{h}", name=f"kT{h}") for h in range(HPC)]
            v_sb = vp.tile([128, S // 128, DH], BF, tag="v")

            # ---- phase A: projections + RoPE ----
            for sc in range(n_sc):
                s0 = sc * QC
                xt = xp.tile([128, EB, QC], BF, tag="xt")
                nc.sync.dma_start(
                    out=xt[:],
                    in_=xT_d.rearrange("(e p) s -> p e s", p=128)[:, :, s0:s0 + QC])
                for h in range(HPC):
                    d0 = h * HD
                    for (wsb, dstT) in ((wq_sb, qT[h]), (wk_sb, kT[h])):
                        ps = ppA.tile([128, QC], F32, tag="pA")
                        for e in range(EB):
                            nc.tensor.matmul(ps[:], wsb[:, e, d0:d0 + HD], xt[:, e, :],
                                             start=(e == 0), stop=(e == EB - 1))
                        m1 = rp.tile([128, QC], F32, tag="m1")
                        m2 = rp.tile([128, QC], F32, tag="m2")
                        nc.vector.tensor_tensor(m2[0:64, :], ps[64:128, :], sin_sb[0:64, s0:s0 + QC], op=ALU.mult)
                        nc.vector.tensor_tensor(m2[64:128, :], ps[0:64, :], sin_sb[64:128, s0:s0 + QC], op=ALU.mult)
                        nc.vector.tensor_tensor(m1[:], ps[:], cos_sb[:, s0:s0 + QC], op=ALU.mult)
                        nc.vector.tensor_tensor(dstT[:, s0:s0 + QC], m1[:], m2[:], op=ALU.add)
                # V for this s-chunk: natural [s, d] layout
                for ss in range(QC // 128):
                    sb = (s0 // 128) + ss
                    ps = ppA.tile([128, DH], F32, tag="pA")
                    for e in range(EB):
                        nc.tensor.matmul(ps[:], xt[:, e, ss * 128:(ss + 1) * 128], wv_sb[:, e, :],
                                         start=(e == 0), stop=(e == EB - 1))
                    nc.scalar.copy(v_sb[:, sb, :], ps[:])

            # ---- phase B: attention (qc outer, h inner), AG + out-proj pipelined ----
            def out_proj(qc):
                ag_sb = wpool.tile([128, EB, QC], BF, tag="w")
                nc.sync.dma_start(out=ag_sb[:], in_=agout[qc].rearrange("(e p) q -> p e q", p=128))
                for qs in range(QC // 128):
                    ps = ppA.tile([128, DH], F32, tag="pA")
                    for e in range(EB):
                        nc.tensor.matmul(ps[:], ag_sb[:, e, qs * 128:(qs + 1) * 128], wo_sb[:, e, :],
                                         start=(e == 0), stop=(e == EB - 1))
                    oc = ocp.tile([128, DH], F32, tag="oc")
                    nc.scalar.copy(oc[:], ps[:])
                    nc.sync.dma_start(out=out_d[qc * QC + qs * 128: qc * QC + (qs + 1) * 128, :], in_=oc[:])

            agout = {}
            for qc in range(n_sc):
                q0 = qc * QC
                agin = dramp.tile([DH, QC], BF, tag="agin")
                for h in range(HPC):
                    nkb = (q0 + QC) // 128  # causal: k blocks up to chunk end
                    ot_ps = ppO.tile([128, QC], F32, tag="pO")
                    sums = ppR.tile([1, QC], F32, tag="pR")
                    for kb in range(nkb):
                        dj = kb - q0 // 128   # >=0 on diagonal chunk
                        o = dj * 128 if dj >= 0 else 0
                        sps = ppS.tile([128, QC], F32, tag="pS")
                        nc.tensor.matmul(sps[:, o:QC], kT[h][:, kb * 128:(kb + 1) * 128],
                                         qT[h][:, q0 + o:q0 + QC],
                                         start=True, stop=True)
                        at = atp.tile([128, QC], BF, tag="at")
                        nc.scalar.activation(at[:, o:QC], sps[:, o:QC], AF.Exp, scale=SCALE)
                        if dj >= 0:
                            nc.vector.tensor_tensor(at[:, o:o + 128], at[:, o:o + 128], mask_sb[:], op=ALU.mult)
                        nc.tensor.matmul(ot_ps[:, o:QC], v_sb[:, kb, h * HD:(h + 1) * HD], at[:, o:QC],
                                         start=(kb == 0), stop=(kb == nkb - 1), skip_group_check=True)
                        nc.tensor.matmul(sums[:, o:QC], ones_sb[:], at[:, o:QC],
                                         start=(kb == 0), stop=(kb == nkb - 1), skip_group_check=True)
                    recip = rnp.tile([1, QC], F32, tag="recip")
                    nc.vector.reciprocal(recip[:], sums[:])
                    rbc = rnp.tile([128, QC], F32, tag="rbc")
                    nc.gpsimd.partition_broadcast(rbc[:], recip[:])
                    ot = otp.tile([128, QC], BF, tag="ot")
                    nc.vector.tensor_tensor(ot[:], ot_ps[:], rbc[:], op=ALU.mult)
                    nc.sync.dma_start(out=agin[h * 128:(h + 1) * 128, :], in_=ot[:])
                ago = dramp.tile([H, QC], BF, tag="agout")
                agout[qc] = ago
                nc.gpsimd.collective_compute(
                    "AllGather", mybir.AluOpType.bypass,
                    ins=[agin[:]], outs=[ago[:]],
                    replica_groups=[[0, 1, 2, 3], [4, 5, 6, 7]],
                )
                if qc >= 1:
                    out_proj(qc - 1)
            out_proj(n_sc - 1)

    nc.compile()
    return nc


def _prep_in_maps(hidden_states, cos, sin, Wq, Wk, Wv, Wo):
    bf = ml_dtypes.bfloat16
    cosT = np.ascontiguousarray(cos[0, 0].T).astype(np.float32)
    sinTs = np.ascontiguousarray(sin[0, 0].T).astype(np.float32).copy()
    sinTs[0:64] *= -1.0
    mask01 = np.triu(np.ones((128, 128), np.float32)).astype(bf)
    in_maps = []
    for c in range(8):
        b, t = c // 4, c % 4
        rows = slice(DH * t, DH * (t + 1))
        in_maps.append({
            "xT": np.ascontiguousarray(hidden_states[b].T).astype(bf),
            "wqT": np.ascontiguousarray(Wq[rows, :].T).astype(bf),
            "wkT": np.ascontiguousarray(Wk[rows, :].T).astype(bf),
            "wvT": np.ascontiguousarray(Wv[rows, :].T).astype(bf),
            "woT": np.ascontiguousarray(Wo[rows, :].T).astype(bf),
            "cosT": cosT,
            "sinTs": sinTs,
            "mask01": mask01,
        })
    return in_maps


def kernel(hidden_states, cos, sin, Wq, Wk, Wv, Wo):
    from concourse.bass_utils import run_bass_kernel_spmd
    if "nc" not in _cached:
        _cached["nc"] = _build()
    nc = _cached["nc"]
    in_maps = _prep_in_maps(hidden_states, cos, sin, Wq, Wk, Wv, Wo)
    res = run_bass_kernel_spmd(nc, in_maps, core_ids=list(range(8)))
    out = np.empty((B, S, H), np.float32)
    for c in range(8):
        b, t = c // 4, c % 4
        out[b, :, DH * t:DH * (t + 1)] = res.results[c]["out"]
    return out
